# revision 24
# baseline (speedup 1.0000x reference)
# Trainium2 Bass kernel for DST_Decoder.
#
# Math reformulation (exact):
#   h  = relu(x @ w1 + b1);  p = h @ w2 + b2                  (pointwise MLP)
#   dx_t = p_t - p_{t-1} (p_{-1}=0);  praw_t = p_t + p_{t-1} = 2*m_t
#   S1_t = p_t;  S2_t = sum_{s<=t} m_s (x) dx_s               (Chen identity)
#   z_t  = cumsum_t[ vec(praw (x) dx) @ (0.5*W1_sig) + dx @ W1_s1 ] + bb1
#   out  = relu(z) @ W2 + bb2
# i.e. contract each timestep's rank-1 outer-product update with W1 FIRST,
# then a cheap 64-wide cumulative scan.  The 0.5 midpoint factor is folded
# into W1_sig on the host so praw needs only an add (done on GPSIMD).
#
# Layout: features on SBUF partitions, time on the free axis; x pre-transposed
# bf16 from the host.  The outer-product tensor O^T (1024, t) is built k-tile
# by k-tile: PE broadcasts rows of praw^T to 128 partitions via a 0/1
# selection matrix (E_r @ praw^T -> 2-bank PSUM tile), DVE multiplies with a
# 4x-stacked bf16 copy of dx^T.  2 of 8 k-tiles per batch are rerouted
# through a Scalar-engine PSUM->SBUF bf16 copy so their multiply runs in the
# DVE 2x bf16 mode (DVE/ACT load balance).  All matmul operands are bf16 and
# every matmul is padded to contraction dim K=128 (K<128 matmuls run ~2x
# slower on TRN2); pad rows hold zero weights and zeroed/finite rhs rows.
# All weights ship in ONE packed DRAM tensor (each dma_start costs ~0.7us of
# serialized Sync-engine descriptor generation, so DMA count is minimized).
# Software-pipelined per-batch loop.
# Sharding: data-parallel over batch, 4 batches per core, weights replicated.

import os
import sys

import numpy as np

for _p in ("/opt/trn_rl_repo",):
    if _p not in sys.path and os.path.isdir(_p):
        sys.path.append(_p)

from concourse import bacc, tile
from concourse import bass_utils
import concourse.mybir as mybir

F32 = mybir.dt.float32
BF16 = mybir.dt.bfloat16

N_CORES = 8
B, L, DIN = 32, 1024, 256
C, HID, DOUT = 32, 64, 128
B_CORE = B // N_CORES                 # 4 batches per core
T = B_CORE * L                        # 4096 time positions per core
KT = (C * C) // 128                   # 8 k-tiles of the outer-product block
ACT_ROUTE = (3, 6)                    # k-tiles whose multiply goes bf16 via ACT

# packed const layout (columns in cw)
CW_W1 = 0                             # 2*HID
CW_W2 = 128                           # C
CW_W1M = 160                          # KT*HID
CW_W1DX = 672                         # HID
CW_E = 736                            # KT*128
CW_W2B = 1760                         # DOUT
CW_COLS = 1888

TRACE = False
LAST_EXEC_NS = None
LAST_PROFILE = None
LAST_TRACE_PATH = None


def build_nc(t_total=T, seq_len=L, chunk=512):
    n_batch = t_total // seq_len      # 4 batches
    cpb = seq_len // chunk            # 2 chunks per batch

    nc = bacc.Bacc(trn_type="TRN2", target_bir_lowering=False, debug=False)

    xTb = nc.dram_tensor("xTb", (n_batch, 128, 2 * seq_len), BF16,
                         kind="ExternalInput").ap()
    cw = nc.dram_tensor("cw", (128, CW_COLS), BF16, kind="ExternalInput").ap()
    cb = nc.dram_tensor("cb", (HID, 3), F32, kind="ExternalInput").ap()
    padrows = nc.dram_tensor("padrows", (97, t_total), BF16,
                             kind="ExternalInput").ap()
    out = nc.dram_tensor("out", (DOUT, t_total), F32, kind="ExternalOutput").ap()

    RELU = mybir.ActivationFunctionType.Relu
    IDENT = mybir.ActivationFunctionType.Identity

    with tile.TileContext(nc) as tc:
        with (
            tc.tile_pool(name="consts", bufs=1) as cpool,
            tc.tile_pool(name="persist", bufs=1) as ppool,
            tc.tile_pool(name="xin", bufs=3) as xpool,
            tc.tile_pool(name="mbs", bufs=2) as mpool,
            tc.tile_pool(name="rhs", bufs=4) as rpool,
            tc.tile_pool(name="zbuf", bufs=2) as zpool,
            tc.tile_pool(name="obuf", bufs=2) as opool,
            tc.tile_pool(name="ps1", bufs=2, space="PSUM") as ps1,
            tc.tile_pool(name="psu", bufs=2, space="PSUM") as psu,
            tc.tile_pool(name="psm", bufs=2, space="PSUM") as psm,
        ):
            # batch-0 x load goes first: the very first matmul waits on it
            xts = {}

            def issue_x(b):
                xt = xpool.tile([128, 2, seq_len], BF16, tag="xt")
                nc.sync.dma_start(out=xt[:], in_=xTb[b])
                xts[b] = xt

            issue_x(0)

            # ---- constants (two DMAs) -------------------------------------
            cw_sb = cpool.tile([128, CW_COLS], BF16, tag="cw")
            nc.sync.dma_start(out=cw_sb[:], in_=cw)
            cb_sb = cpool.tile([HID, 3], F32, tag="cb")
            nc.sync.dma_start(out=cb_sb[:], in_=cb)
            b1_sb = cb_sb[:, 0:1]
            b2_sb = cb_sb[0:C, 1:2]
            bb1_sb = cb_sb[:, 2:3]

            # ---- persistent activations (padded to 128 partitions) --------
            pT = ppool.tile([C, t_total], F32, tag="pT")
            dxT = ppool.tile([C, t_total], F32, tag="dxT")
            dx4 = ppool.tile([128, t_total], BF16, tag="dx4")
            mTr = ppool.tile([128, t_total], BF16, tag="mTr")
            uT = ppool.tile([HID, t_total], BF16, tag="uT")
            aT = ppool.tile([128, t_total], BF16, tag="aT")
            hsbP = ppool.tile([128, 2, chunk], BF16, tag="hsbP")
            # hsbP pad rows are read by the first pps matmul -> load before
            # phase1(0); the rest land during early compute.
            nc.sync.dma_start(out=hsbP[HID:128, :, :],
                              in_=padrows[1:1 + 128 - HID, 0:2 * chunk])

            def phase1(b):
                # pointwise MLP for both chunks of batch b (x prefetched)
                xt = xts.pop(b)
                for ci in range(cpb):
                    c = b * cpb + ci
                    cs = slice(c * chunk, (c + 1) * chunk)
                    lo = slice(ci * chunk, (ci + 1) * chunk)
                    hps = ps1.tile([HID, chunk], F32, tag="ps1t")
                    for k in range(2):
                        nc.tensor.matmul(hps[:],
                                         cw_sb[:, CW_W1 + k * HID:
                                               CW_W1 + (k + 1) * HID],
                                         xt[:, k, lo],
                                         start=(k == 0), stop=(k == 1))
                    nc.scalar.activation(hsbP[0:HID, ci, :], hps[:], RELU,
                                         bias=b1_sb, scale=1.0)
                    pps = ps1.tile([C, chunk], F32, tag="ps1t")
                    nc.tensor.matmul(pps[:], cw_sb[:, CW_W2:CW_W2 + C],
                                     hsbP[:, ci, :], start=True, stop=True)
                    nc.scalar.add(pT[:, cs], pps[:], b2_sb)

            def phase2(b):
                t0 = b * seq_len
                bs = slice(t0, t0 + seq_len)
                nc.gpsimd.tensor_copy(dxT[:, t0:t0 + 1], pT[:, t0:t0 + 1])
                nc.gpsimd.tensor_sub(
                    dxT[:, t0 + 1:t0 + seq_len],
                    pT[:, t0 + 1:t0 + seq_len],
                    pT[:, t0:t0 + seq_len - 1],
                )
                nc.scalar.activation(dx4[0:C, bs], dxT[:, bs], IDENT,
                                     bias=0.0, scale=1.0)
                nc.sync.dma_start(out=dx4[C:2 * C, bs], in_=dx4[0:C, bs])
                nc.sync.dma_start(out=dx4[2 * C:4 * C, bs], in_=dx4[0:2 * C, bs])

            def phase2_mTr(b):
                # praw = p_t + p_{t-1} (= 2*m; the 0.5 lives in W1m).  On DVE;
                # injected mid-way into the previous batch's multiply stream so
                # the DVE never head-of-line blocks on the pT chain.
                t0 = b * seq_len
                nc.vector.tensor_copy(mTr[0:C, t0:t0 + 1], pT[:, t0:t0 + 1])
                nc.vector.tensor_add(
                    mTr[0:C, t0 + 1:t0 + seq_len],
                    pT[:, t0 + 1:t0 + seq_len],
                    pT[:, t0:t0 + seq_len - 1],
                )

            def phase3(b, inject=None):
                # outer-product build + contraction with W1 for batch b
                t0 = b * seq_len
                bs = slice(t0, t0 + seq_len)
                chunks = [b * cpb + i for i in range(cpb)]
                rhss = []
                for r in range(KT):
                    if r == 2 and inject is not None:
                        inject()
                    mb = psm.tile([128, seq_len], F32, tag="mb")
                    for i, c in enumerate(chunks):
                        cs = slice(c * chunk, (c + 1) * chunk)
                        nc.tensor.matmul(
                            mb[:, i * chunk:(i + 1) * chunk],
                            cw_sb[:, CW_E + r * 128:CW_E + (r + 1) * 128],
                            mTr[:, cs],
                            start=True, stop=True,
                        )
                    rhsb = rpool.tile([128, seq_len], BF16, tag="rhsb")
                    if r in ACT_ROUTE:
                        mbs = mpool.tile([128, seq_len], BF16, tag="mbs")
                        nc.scalar.copy(mbs[:], mb[:])
                        nc.vector.tensor_mul(rhsb[:], mbs[:], dx4[:, bs])
                    else:
                        nc.vector.tensor_mul(rhsb[:], mb[:], dx4[:, bs])
                    rhss.append(rhsb)
                upss = []
                for _ in chunks:
                    ups = psu.tile([HID, chunk], F32, tag="ups")
                    upss.append(ups)
                for r in range(KT):
                    for i in range(cpb):
                        nc.tensor.matmul(
                            upss[i][:],
                            cw_sb[:, CW_W1M + r * HID:CW_W1M + (r + 1) * HID],
                            rhss[r][:, i * chunk:(i + 1) * chunk],
                            start=(r == 0), stop=False,
                        )
                for i, c in enumerate(chunks):
                    cs = slice(c * chunk, (c + 1) * chunk)
                    nc.tensor.matmul(upss[i][:],
                                     cw_sb[:, CW_W1DX:CW_W1DX + HID],
                                     dx4[:, cs], start=False, stop=True)
                for i, c in enumerate(chunks):
                    cs = slice(c * chunk, (c + 1) * chunk)
                    nc.scalar.copy(uT[:, cs], upss[i][:])

            def phase4(b):
                t0 = b * seq_len
                bs = slice(t0, t0 + seq_len)
                zb = zpool.tile([HID, seq_len], BF16, tag="zb")
                nc.vector.tensor_tensor_scan(
                    zb[:], uT[:, bs], uT[:, bs], 0.0,
                    op0=mybir.AluOpType.add, op1=mybir.AluOpType.bypass,
                )
                nc.scalar.activation(aT[0:HID, bs], zb[:], RELU,
                                     bias=bb1_sb, scale=1.0)

            def phase5(b):
                t0 = b * seq_len
                bs = slice(t0, t0 + seq_len)
                osb = opool.tile([DOUT, seq_len], F32, tag="osb")
                for ci in range(cpb):
                    c = b * cpb + ci
                    cs = slice(c * chunk, (c + 1) * chunk)
                    ops = ps1.tile([DOUT, chunk], F32, tag="ps1t")
                    nc.tensor.matmul(ops[:], cw_sb[:, CW_W2B:CW_W2B + DOUT],
                                     aT[:, cs], start=True, stop=True)
                    nc.scalar.copy(osb[:, ci * chunk:(ci + 1) * chunk], ops[:])
                nc.sync.dma_start(out=out[:, bs], in_=osb[:])

            def phase45_tail(b):
                # chunk-granular drain of the final batch: scan/relu/head MM
                # pipeline per 512 chunk instead of batch-serial
                t0 = b * seq_len
                zb = zpool.tile([HID, seq_len], BF16, tag="zb")
                for ci in range(cpb):
                    lo = slice(ci * chunk, (ci + 1) * chunk)
                    cs = slice(t0 + ci * chunk, t0 + (ci + 1) * chunk)
                    init = 0.0 if ci == 0 else zb[:, ci * chunk - 1:ci * chunk]
                    nc.vector.tensor_tensor_scan(
                        zb[:, lo], uT[:, cs], uT[:, cs], init,
                        op0=mybir.AluOpType.add, op1=mybir.AluOpType.bypass,
                    )
                    nc.scalar.activation(aT[0:HID, cs], zb[:, lo], RELU,
                                         bias=bb1_sb, scale=1.0)
                    ops = ps1.tile([DOUT, chunk], F32, tag="ps1t")
                    nc.tensor.matmul(ops[:], cw_sb[:, CW_W2B:CW_W2B + DOUT],
                                     aT[:, cs], start=True, stop=True)
                    osb = opool.tile([DOUT, chunk], F32, tag="osbt")
                    nc.scalar.copy(osb[:], ops[:])
                    nc.sync.dma_start(out=out[:, cs], in_=osb[:])

            # ---- software-pipelined schedule (phase3 lags phase1 by 2) ----
            for it in range(n_batch + 3):
                if it < n_batch:
                    phase1(it)
                    phase2(it)
                if it <= 1 and it < n_batch:
                    phase2_mTr(it)
                if it == 0:
                    # pad rows: mTr read by phase3(0), aT by phase5(0);
                    # emitted after batch-0 work so they don't delay it.
                    nc.sync.dma_start(out=mTr[C:128, :], in_=padrows[1:97, :])
                    nc.sync.dma_start(out=aT[HID:128, :], in_=padrows[0:64, :])
                if 2 <= it <= n_batch + 1:
                    b3 = it - 2
                    inj = (lambda b2=it: phase2_mTr(b2)) if it < n_batch else None
                    phase3(b3, inject=inj)
                    if it == n_batch + 1:
                        phase45_tail(b3)
                    else:
                        phase4(b3)
                if it >= 3 and it - 3 != n_batch - 1:
                    phase5(it - 3)
                if it + 1 < n_batch:
                    issue_x(it + 1)

    nc.compile()
    return nc


def host_prep_shared(w1, b1, w2, b2, W1, bb1, W2, bb2):
    import ml_dtypes
    bf = ml_dtypes.bfloat16
    f = np.float32

    cwm = np.zeros((128, CW_COLS), f)
    # w1 interleaved k-major: col k*HID+n holds w1[k*128+p, n]
    cwm[:, CW_W1:CW_W1 + 2 * HID] = (
        np.asarray(w1, f).reshape(2, 128, HID).transpose(1, 0, 2).reshape(128, -1))
    cwm[0:HID, CW_W2:CW_W2 + C] = np.asarray(w2, f)
    # 0.5 midpoint factor folded into W1_sig
    cwm[:, CW_W1M:CW_W1M + KT * HID] = (
        0.5 * np.asarray(W1[C:], f).reshape(KT, 128, HID)
        .transpose(1, 0, 2).reshape(128, -1))
    cwm[0:C, CW_W1DX:CW_W1DX + HID] = np.asarray(W1[:C], f)
    for r in range(KT):
        for q in range(128):
            cwm[4 * r + q // 32, CW_E + 128 * r + q] = 1.0
    cwm[0:HID + 1, CW_W2B:CW_W2B + DOUT] = np.vstack([np.asarray(W2, f),
                                                      np.asarray(bb2, f)[None, :]])

    cbm = np.zeros((HID, 3), f)
    cbm[:, 0] = np.asarray(b1, f)
    cbm[0:C, 1] = np.asarray(b2, f)
    cbm[:, 2] = np.asarray(bb1, f)

    pad = np.zeros((97, T), f)
    pad[0, :] = 1.0
    return {
        "cw": cwm.astype(bf),
        "cb": cbm,
        "padrows": pad.astype(bf),
    }


_NC_CACHE = {}


def _get_nc():
    key = "full"
    if key not in _NC_CACHE:
        _NC_CACHE[key] = build_nc()
    return _NC_CACHE[key]


def kernel(x, w1, b1, w2, b2, W1, bb1, W2, bb2):
    global LAST_EXEC_NS, LAST_PROFILE, LAST_TRACE_PATH
    import ml_dtypes
    bf = ml_dtypes.bfloat16
    nc = _get_nc()
    shared = host_prep_shared(w1, b1, w2, b2, W1, bb1, W2, bb2)
    xbf = np.ascontiguousarray(x, np.float32).astype(bf)
    n_batch = T // L
    in_maps = []
    for core in range(N_CORES):
        xc = xbf[core * B_CORE:(core + 1) * B_CORE].reshape(T, DIN)
        # (256, T) -> (n_batch, 128, 2*L): [b][p][k*L+t] = xT[k*128+p, b*L+t]
        xT = xc.T.reshape(2, 128, n_batch, L)
        xTb = np.ascontiguousarray(xT.transpose(2, 1, 0, 3)).reshape(
            n_batch, 128, 2 * L)
        m = dict(shared)
        m["xTb"] = np.ascontiguousarray(xTb)
        in_maps.append(m)
    try:
        res = bass_utils.run_bass_kernel_spmd(
            nc, in_maps, core_ids=list(range(N_CORES)), trace=TRACE,
        )
    except Exception:
        if not TRACE:
            raise
        res = bass_utils.run_bass_kernel_spmd(
            nc, in_maps, core_ids=list(range(N_CORES)), trace=False,
        )
    LAST_EXEC_NS = res.exec_time_ns
    LAST_PROFILE = res.profile_json
    LAST_TRACE_PATH = (res.instructions_and_trace or (None, None))[1]
    outs = [np.ascontiguousarray(res.results[i]["out"].T).reshape(B_CORE, L, DOUT)
            for i in range(N_CORES)]
    return np.concatenate(outs, axis=0)


# revision 27
# speedup vs baseline: 1.0026x; 1.0026x over previous
# Trainium2 Bass kernel for DST_Decoder.
#
# Math reformulation (exact):
#   h  = relu(x @ w1 + b1);  p = h @ w2 + b2                  (pointwise MLP)
#   dx_t = p_t - p_{t-1} (p_{-1}=0);  praw_t = p_t + p_{t-1} = 2*m_t
#   S1_t = p_t;  S2_t = sum_{s<=t} m_s (x) dx_s               (Chen identity)
#   z_t  = cumsum_t[ vec(praw (x) dx) @ (0.5*W1_sig) + dx @ W1_s1 ] + bb1
#   out  = relu(z) @ W2 + bb2
# i.e. contract each timestep's rank-1 outer-product update with W1 FIRST,
# then a cheap 64-wide cumulative scan.  The 0.5 midpoint factor is folded
# into W1_sig on the host so praw needs only an add (done on GPSIMD).
#
# Layout: features on SBUF partitions, time on the free axis; x pre-transposed
# bf16 from the host.  The outer-product tensor O^T (1024, t) is built k-tile
# by k-tile: PE broadcasts rows of praw^T to 128 partitions via a 0/1
# selection matrix (E_r @ praw^T -> 2-bank PSUM tile), DVE multiplies with a
# 4x-stacked bf16 copy of dx^T.  2 of 8 k-tiles per batch are rerouted
# through a Scalar-engine PSUM->SBUF bf16 copy so their multiply runs in the
# DVE 2x bf16 mode (DVE/ACT load balance).  All matmul operands are bf16 and
# every matmul is padded to contraction dim K=128 (K<128 matmuls run ~2x
# slower on TRN2); pad rows hold zero weights and zeroed/finite rhs rows.
# All weights ship in ONE packed DRAM tensor (each dma_start costs ~0.7us of
# serialized Sync-engine descriptor generation, so DMA count is minimized).
# Software-pipelined per-batch loop.
# Sharding: data-parallel over batch, 4 batches per core, weights replicated.

import os
import sys

import numpy as np

for _p in ("/opt/trn_rl_repo",):
    if _p not in sys.path and os.path.isdir(_p):
        sys.path.append(_p)

from concourse import bacc, tile
from concourse import bass_utils
import concourse.mybir as mybir

F32 = mybir.dt.float32
BF16 = mybir.dt.bfloat16

N_CORES = 8
B, L, DIN = 32, 1024, 256
C, HID, DOUT = 32, 64, 128
B_CORE = B // N_CORES                 # 4 batches per core
T = B_CORE * L                        # 4096 time positions per core
KT = (C * C) // 128                   # 8 k-tiles of the outer-product block
ACT_ROUTE = (3, 6)                    # k-tiles whose multiply goes bf16 via ACT

# packed const layout (columns in cw)
CW_W1 = 0                             # 2*HID
CW_W2 = 128                           # C
CW_W1M = 160                          # KT*HID
CW_W1DX = 672                         # HID
CW_E = 736                            # KT*128
CW_W2B = 1760                         # DOUT
CW_COLS = 1888

TRACE = False
LAST_EXEC_NS = None
LAST_PROFILE = None
LAST_TRACE_PATH = None


def build_nc(t_total=T, seq_len=L, chunk=512):
    n_batch = t_total // seq_len      # 4 batches
    cpb = seq_len // chunk            # 2 chunks per batch

    nc = bacc.Bacc(trn_type="TRN2", target_bir_lowering=False, debug=False)

    xTb = nc.dram_tensor("xTb", (n_batch, 128, 2 * seq_len), BF16,
                         kind="ExternalInput").ap()
    cw = nc.dram_tensor("cw", (128, CW_COLS), BF16, kind="ExternalInput").ap()
    cb = nc.dram_tensor("cb", (HID, 3), F32, kind="ExternalInput").ap()
    padrows = nc.dram_tensor("padrows", (97, t_total), BF16,
                             kind="ExternalInput").ap()
    out = nc.dram_tensor("out", (DOUT, t_total), F32, kind="ExternalOutput").ap()

    RELU = mybir.ActivationFunctionType.Relu
    IDENT = mybir.ActivationFunctionType.Identity

    with tile.TileContext(nc) as tc:
        with (
            tc.tile_pool(name="consts", bufs=1) as cpool,
            tc.tile_pool(name="persist", bufs=1) as ppool,
            tc.tile_pool(name="xin", bufs=3) as xpool,
            tc.tile_pool(name="mbs", bufs=3) as mpool,
            tc.tile_pool(name="rhs", bufs=6) as rpool,
            tc.tile_pool(name="zbuf", bufs=2) as zpool,
            tc.tile_pool(name="obuf", bufs=2) as opool,
            tc.tile_pool(name="ps1", bufs=2, space="PSUM") as ps1,
            tc.tile_pool(name="psu", bufs=2, space="PSUM") as psu,
            tc.tile_pool(name="psm", bufs=2, space="PSUM") as psm,
        ):
            # batch-0 x load goes first: the very first matmul waits on it
            xts = {}

            def issue_x(b):
                xt = xpool.tile([128, 2, seq_len], BF16, tag="xt")
                nc.sync.dma_start(out=xt[:], in_=xTb[b])
                xts[b] = xt

            issue_x(0)

            # ---- constants (two DMAs) -------------------------------------
            cw_sb = cpool.tile([128, CW_COLS], BF16, tag="cw")
            nc.sync.dma_start(out=cw_sb[:], in_=cw)
            cb_sb = cpool.tile([HID, 3], F32, tag="cb")
            nc.sync.dma_start(out=cb_sb[:], in_=cb)
            b1_sb = cb_sb[:, 0:1]
            b2_sb = cb_sb[0:C, 1:2]
            bb1_sb = cb_sb[:, 2:3]

            # ---- persistent activations (padded to 128 partitions) --------
            pT = ppool.tile([C, t_total], F32, tag="pT")
            dxT = ppool.tile([C, t_total], F32, tag="dxT")
            dx4 = ppool.tile([128, t_total], BF16, tag="dx4")
            mTr = ppool.tile([128, t_total], BF16, tag="mTr")
            uT = ppool.tile([HID, t_total], BF16, tag="uT")
            aT = ppool.tile([128, t_total], BF16, tag="aT")
            hsbP = ppool.tile([128, 2, chunk], BF16, tag="hsbP")
            # hsbP pad rows are read by the first pps matmul -> load before
            # phase1(0); the rest land during early compute.
            nc.sync.dma_start(out=hsbP[HID:128, :, :],
                              in_=padrows[1:1 + 128 - HID, 0:2 * chunk])

            def phase1(b):
                # pointwise MLP for both chunks of batch b (x prefetched)
                xt = xts.pop(b)
                for ci in range(cpb):
                    c = b * cpb + ci
                    cs = slice(c * chunk, (c + 1) * chunk)
                    lo = slice(ci * chunk, (ci + 1) * chunk)
                    hps = ps1.tile([HID, chunk], F32, tag="ps1t")
                    for k in range(2):
                        nc.tensor.matmul(hps[:],
                                         cw_sb[:, CW_W1 + k * HID:
                                               CW_W1 + (k + 1) * HID],
                                         xt[:, k, lo],
                                         start=(k == 0), stop=(k == 1))
                    nc.scalar.activation(hsbP[0:HID, ci, :], hps[:], RELU,
                                         bias=b1_sb, scale=1.0)
                    pps = ps1.tile([C, chunk], F32, tag="ps1t")
                    nc.tensor.matmul(pps[:], cw_sb[:, CW_W2:CW_W2 + C],
                                     hsbP[:, ci, :], start=True, stop=True)
                    nc.scalar.add(pT[:, cs], pps[:], b2_sb)

            def phase2(b):
                t0 = b * seq_len
                bs = slice(t0, t0 + seq_len)
                nc.gpsimd.tensor_copy(dxT[:, t0:t0 + 1], pT[:, t0:t0 + 1])
                nc.gpsimd.tensor_sub(
                    dxT[:, t0 + 1:t0 + seq_len],
                    pT[:, t0 + 1:t0 + seq_len],
                    pT[:, t0:t0 + seq_len - 1],
                )
                nc.scalar.activation(dx4[0:C, bs], dxT[:, bs], IDENT,
                                     bias=0.0, scale=1.0)
                nc.sync.dma_start(out=dx4[C:2 * C, bs], in_=dx4[0:C, bs])
                nc.sync.dma_start(out=dx4[2 * C:4 * C, bs], in_=dx4[0:2 * C, bs])

            def phase2_mTr(b):
                # praw = p_t + p_{t-1} (= 2*m; the 0.5 lives in W1m).  On DVE;
                # injected mid-way into the previous batch's multiply stream so
                # the DVE never head-of-line blocks on the pT chain.
                t0 = b * seq_len
                nc.vector.tensor_copy(mTr[0:C, t0:t0 + 1], pT[:, t0:t0 + 1])
                nc.vector.tensor_add(
                    mTr[0:C, t0 + 1:t0 + seq_len],
                    pT[:, t0 + 1:t0 + seq_len],
                    pT[:, t0:t0 + seq_len - 1],
                )

            def phase3(b, inject=None):
                # outer-product build + contraction with W1 for batch b.
                # E-broadcast and contraction matmuls are braided (mains for
                # k-tile r emitted right after the E matmuls of r+1) so the PE
                # stream stays dense while the DVE multiplies pace the middle.
                t0 = b * seq_len
                bs = slice(t0, t0 + seq_len)
                chunks = [b * cpb + i for i in range(cpb)]
                rhss = []
                upss = []
                for _ in chunks:
                    ups = psu.tile([HID, chunk], F32, tag="ups")
                    upss.append(ups)

                def mains(r):
                    for i in range(cpb):
                        nc.tensor.matmul(
                            upss[i][:],
                            cw_sb[:, CW_W1M + r * HID:CW_W1M + (r + 1) * HID],
                            rhss[r][:, i * chunk:(i + 1) * chunk],
                            start=(r == 0), stop=False,
                        )

                for r in range(KT):
                    if r == 2 and inject is not None:
                        inject()
                    mb = psm.tile([128, seq_len], F32, tag="mb")
                    for i, c in enumerate(chunks):
                        cs = slice(c * chunk, (c + 1) * chunk)
                        nc.tensor.matmul(
                            mb[:, i * chunk:(i + 1) * chunk],
                            cw_sb[:, CW_E + r * 128:CW_E + (r + 1) * 128],
                            mTr[:, cs],
                            start=True, stop=True,
                        )
                    rhsb = rpool.tile([128, seq_len], BF16, tag="rhsb")
                    if r in ACT_ROUTE:
                        mbs = mpool.tile([128, seq_len], BF16, tag="mbs")
                        nc.scalar.copy(mbs[:], mb[:])
                        nc.vector.tensor_mul(rhsb[:], mbs[:], dx4[:, bs])
                    else:
                        nc.vector.tensor_mul(rhsb[:], mb[:], dx4[:, bs])
                    rhss.append(rhsb)
                    if r >= 1:
                        mains(r - 1)
                mains(KT - 1)
                for i, c in enumerate(chunks):
                    cs = slice(c * chunk, (c + 1) * chunk)
                    nc.tensor.matmul(upss[i][:],
                                     cw_sb[:, CW_W1DX:CW_W1DX + HID],
                                     dx4[:, cs], start=False, stop=True)
                for i, c in enumerate(chunks):
                    cs = slice(c * chunk, (c + 1) * chunk)
                    nc.scalar.copy(uT[:, cs], upss[i][:])

            def phase4(b):
                t0 = b * seq_len
                bs = slice(t0, t0 + seq_len)
                zb = zpool.tile([HID, seq_len], BF16, tag="zb")
                nc.vector.tensor_tensor_scan(
                    zb[:], uT[:, bs], uT[:, bs], 0.0,
                    op0=mybir.AluOpType.add, op1=mybir.AluOpType.bypass,
                )
                nc.scalar.activation(aT[0:HID, bs], zb[:], RELU,
                                     bias=bb1_sb, scale=1.0)

            def phase5(b):
                t0 = b * seq_len
                bs = slice(t0, t0 + seq_len)
                osb = opool.tile([DOUT, seq_len], F32, tag="osb")
                for ci in range(cpb):
                    c = b * cpb + ci
                    cs = slice(c * chunk, (c + 1) * chunk)
                    ops = ps1.tile([DOUT, chunk], F32, tag="ps1t")
                    nc.tensor.matmul(ops[:], cw_sb[:, CW_W2B:CW_W2B + DOUT],
                                     aT[:, cs], start=True, stop=True)
                    nc.scalar.copy(osb[:, ci * chunk:(ci + 1) * chunk], ops[:])
                nc.sync.dma_start(out=out[:, bs], in_=osb[:])

            def phase45_tail(b):
                # chunk-granular drain of the final batch: scan/relu/head MM
                # pipeline per 512 chunk instead of batch-serial
                t0 = b * seq_len
                zb = zpool.tile([HID, seq_len], BF16, tag="zb")
                for ci in range(cpb):
                    lo = slice(ci * chunk, (ci + 1) * chunk)
                    cs = slice(t0 + ci * chunk, t0 + (ci + 1) * chunk)
                    init = 0.0 if ci == 0 else zb[:, ci * chunk - 1:ci * chunk]
                    nc.vector.tensor_tensor_scan(
                        zb[:, lo], uT[:, cs], uT[:, cs], init,
                        op0=mybir.AluOpType.add, op1=mybir.AluOpType.bypass,
                    )
                    nc.scalar.activation(aT[0:HID, cs], zb[:, lo], RELU,
                                         bias=bb1_sb, scale=1.0)
                    ops = ps1.tile([DOUT, chunk], F32, tag="ps1t")
                    nc.tensor.matmul(ops[:], cw_sb[:, CW_W2B:CW_W2B + DOUT],
                                     aT[:, cs], start=True, stop=True)
                    osb = opool.tile([DOUT, chunk], F32, tag="osbt")
                    nc.scalar.copy(osb[:], ops[:])
                    nc.sync.dma_start(out=out[:, cs], in_=osb[:])

            # ---- software-pipelined schedule ------------------------------
            for it in range(n_batch + 2):
                if it < n_batch:
                    phase1(it)
                    phase2(it)
                if it == 0:
                    phase2_mTr(0)
                    # pad rows: mTr read by phase3(0), aT by phase5(0);
                    # emitted after batch-0 work so they don't delay it.
                    nc.sync.dma_start(out=mTr[C:128, :], in_=padrows[1:97, :])
                    nc.sync.dma_start(out=aT[HID:128, :], in_=padrows[0:64, :])
                if 1 <= it <= n_batch:
                    inj = (lambda b2=it: phase2_mTr(b2)) if it < n_batch else None
                    phase3(it - 1, inject=inj)
                    if it == n_batch:
                        phase45_tail(it - 1)
                    else:
                        phase4(it - 1)
                if it >= 2 and it - 2 != n_batch - 1:
                    phase5(it - 2)
                if it + 1 < n_batch:
                    issue_x(it + 1)

    nc.compile()
    return nc


def host_prep_shared(w1, b1, w2, b2, W1, bb1, W2, bb2):
    import ml_dtypes
    bf = ml_dtypes.bfloat16
    f = np.float32

    cwm = np.zeros((128, CW_COLS), f)
    # w1 interleaved k-major: col k*HID+n holds w1[k*128+p, n]
    cwm[:, CW_W1:CW_W1 + 2 * HID] = (
        np.asarray(w1, f).reshape(2, 128, HID).transpose(1, 0, 2).reshape(128, -1))
    cwm[0:HID, CW_W2:CW_W2 + C] = np.asarray(w2, f)
    # 0.5 midpoint factor folded into W1_sig
    cwm[:, CW_W1M:CW_W1M + KT * HID] = (
        0.5 * np.asarray(W1[C:], f).reshape(KT, 128, HID)
        .transpose(1, 0, 2).reshape(128, -1))
    cwm[0:C, CW_W1DX:CW_W1DX + HID] = np.asarray(W1[:C], f)
    for r in range(KT):
        for q in range(128):
            cwm[4 * r + q // 32, CW_E + 128 * r + q] = 1.0
    cwm[0:HID + 1, CW_W2B:CW_W2B + DOUT] = np.vstack([np.asarray(W2, f),
                                                      np.asarray(bb2, f)[None, :]])

    cbm = np.zeros((HID, 3), f)
    cbm[:, 0] = np.asarray(b1, f)
    cbm[0:C, 1] = np.asarray(b2, f)
    cbm[:, 2] = np.asarray(bb1, f)

    pad = np.zeros((97, T), f)
    pad[0, :] = 1.0
    return {
        "cw": cwm.astype(bf),
        "cb": cbm,
        "padrows": pad.astype(bf),
    }


_NC_CACHE = {}


def _get_nc():
    key = "full"
    if key not in _NC_CACHE:
        _NC_CACHE[key] = build_nc()
    return _NC_CACHE[key]


def kernel(x, w1, b1, w2, b2, W1, bb1, W2, bb2):
    global LAST_EXEC_NS, LAST_PROFILE, LAST_TRACE_PATH
    import ml_dtypes
    bf = ml_dtypes.bfloat16
    nc = _get_nc()
    shared = host_prep_shared(w1, b1, w2, b2, W1, bb1, W2, bb2)
    xbf = np.ascontiguousarray(x, np.float32).astype(bf)
    n_batch = T // L
    in_maps = []
    for core in range(N_CORES):
        xc = xbf[core * B_CORE:(core + 1) * B_CORE].reshape(T, DIN)
        # (256, T) -> (n_batch, 128, 2*L): [b][p][k*L+t] = xT[k*128+p, b*L+t]
        xT = xc.T.reshape(2, 128, n_batch, L)
        xTb = np.ascontiguousarray(xT.transpose(2, 1, 0, 3)).reshape(
            n_batch, 128, 2 * L)
        m = dict(shared)
        m["xTb"] = np.ascontiguousarray(xTb)
        in_maps.append(m)
    try:
        res = bass_utils.run_bass_kernel_spmd(
            nc, in_maps, core_ids=list(range(N_CORES)), trace=TRACE,
        )
    except Exception:
        if not TRACE:
            raise
        res = bass_utils.run_bass_kernel_spmd(
            nc, in_maps, core_ids=list(range(N_CORES)), trace=False,
        )
    LAST_EXEC_NS = res.exec_time_ns
    LAST_PROFILE = res.profile_json
    LAST_TRACE_PATH = (res.instructions_and_trace or (None, None))[1]
    outs = [np.ascontiguousarray(res.results[i]["out"].T).reshape(B_CORE, L, DOUT)
            for i in range(N_CORES)]
    return np.concatenate(outs, axis=0)
